# revision 1
# baseline (speedup 1.0000x reference)
"""Trainium2 Bass kernel for nn_EventTempRel_HGRU_static (hyperbolic GRU), v2.

Layout: z/r gates partition-stacked ([64,*]: z rows 0-31, r rows 32-63),
h-lane work at parts 0-31.  P-records live in SBUF (no per-step DMA).
Tiny scalar coefficient ops alternate Act/DVE (38-57ns/op); wide ops on
DVE/Pool; dots via asq-accum (Act) or TT+grouped-reduce (DVE); matmuls,
transposes and partition shuffles/dups on PE.
"""
import numpy as np

F32 = np.float32

TAU_MV = (0.3616372627630415, -0.460824836532626, 0.7876404160046405)
TAU_PW = (0.3575593089268115, -0.4659240032983483, 0.7829142723793893)
TAU_PRE = (0.3457879813610502, -0.4806491635079507, 0.7689669798854355)
PHI_ST = (0.4647741909888864, 0.3581349612622114, 0.8717410381896435)
PHI_LOG = (0.49913475137469127, 0.33018529682360925, 0.8910181465187239)
PHI_HEAD = (0.4769722816867955, 0.3481376015135419, 0.8788082074322899)
PSI_G = (0.01980701895330672, -1.059545853653833, 0.8975165215601002)

B, T, DIN, H, DOUT, C = 256, 128, 768, 128, 64, 4
NC_N = 8
BL = B // NC_N
EPS = 1e-5
PF = 132          # per-lane P record: m0(128) | tau | q | pb | pp

_CACHE = {}
_pending_host = {}


def _split_multiwait(nc):
    import concourse.mybir as mybir
    import bass_rust
    for fn in nc.m.functions:
        for blk in fn.blocks:
            newinsts = []
            changed = False
            for inst in blk.instructions:
                si = inst.sync_info
                waits = list(si.on_wait) if si and si.on_wait else []
                if len(waits) > 1:
                    changed = True
                    for k, w in enumerate(waits[:-1]):
                        ev = mybir.InstEventSemaphore(
                            name=f"{inst.name}-w{k}", engine=inst.engine,
                            ins=[], outs=[],
                            sync_info=bass_rust.SyncInfo(on_wait=[w], on_update=[]))
                        newinsts.append(ev)
                    inst.sync_info = bass_rust.SyncInfo(on_wait=[waits[-1]],
                                                        on_update=si.on_update)
                newinsts.append(inst)
            if changed:
                blk.instructions = newinsts


def _make_tc_class():
    from concourse.tile import TileContext
    import bass_rust
    from bass_rust import ScopedClock

    class SplitDrainTC(TileContext):
        def _drain_and_barrier(self, tick_clock, wait_clock):
            nop = self.nc.vector.engine_nop()
            wait_clock.add_sem_waits(nop.ins,
                                     ScopedClock({None: tick_clock.global_clock}))
            si = nop.ins.sync_info
            waits = list(si.on_wait) if si and si.on_wait else []
            if len(waits) > 1:
                nop.ins.sync_info = bass_rust.SyncInfo(on_wait=waits[:1],
                                                       on_update=si.on_update)
                for w in waits[1:]:
                    n2 = self.nc.vector.engine_nop()
                    n2.ins.sync_info = bass_rust.SyncInfo(on_wait=[w], on_update=[])
            self.nc.sync.drain()
            self.nc.all_engine_barrier()
            popped = self.nc._tile_sem_poison_stack.pop()
            assert popped is self._sem_poison
            self.nc.clear_and_free_semaphores(list(self.sems.allocated().values()))
            self.nc.all_engine_barrier()

    return SplitDrainTC


def _build_program():
    import concourse.bass as bass
    import concourse.mybir as mybir
    TileContext = _make_tc_class()

    AF = mybir.ActivationFunctionType
    AL = mybir.AluOpType
    f32 = mybir.dt.float32
    i32 = mybir.dt.int32

    nc = bass.Bass()
    host = _pending_host

    # ---------------- DRAM I/O ----------------
    xseq = nc.dram_tensor("xseq", [BL, T, DIN], f32, kind="ExternalInput")
    m1d = nc.dram_tensor("m1d", [BL, T], f32, kind="ExternalInput")
    m2d = nc.dram_tensor("m2d", [BL, T], f32, kind="ExternalInput")
    cidd = nc.dram_tensor("cidd", [BL, 1], i32, kind="ExternalInput")
    identd = nc.dram_tensor("identd", [128, 128], f32, kind="ExternalInput")
    wzxd = nc.dram_tensor("wzxd", [128, 129], f32, kind="ExternalInput")
    wrxd = nc.dram_tensor("wrxd", [128, 129], f32, kind="ExternalInput")
    whxd = nc.dram_tensor("whxd", [128, 129], f32, kind="ExternalInput")
    utxd = nc.dram_tensor("utxd", [768, 387], f32, kind="ExternalInput")
    shufd = nc.dram_tensor("shufd", [64, 96], f32, kind="ExternalInput")  # Sr32 | Sdup
    bzrwd = nc.dram_tensor("bzrwd", [64, 128], f32, kind="ExternalInput")
    bhd = nc.dram_tensor("bhd", [BL, 128], f32, kind="ExternalInput")
    n2bd = nc.dram_tensor("n2bd", [64, 1], f32, kind="ExternalInput")
    wfuvd = nc.dram_tensor("wfuvd", [128, 128], f32, kind="ExternalInput")
    ctabd = nc.dram_tensor("ctabd", [20, DOUT], f32, kind="ExternalInput")
    pmld = nc.dram_tensor("pmld", [BL, C * DOUT], f32, kind="ExternalInput")
    auld = nc.dram_tensor("auld", [BL, C * DOUT], f32, kind="ExternalInput")
    clcd = nc.dram_tensor("clcd", [BL, 4 * C], f32, kind="ExternalInput")
    bffd = nc.dram_tensor("bffd", [BL, DOUT], f32, kind="ExternalInput")
    bdhd = nc.dram_tensor("bdhd", [BL, DOUT], f32, kind="ExternalInput")
    outd = nc.dram_tensor("out", [BL, C], f32, kind="ExternalOutput")
    import os as _os
    _dbg = _os.environ.get("HGRU_DBG_HIST") == "1"
    histo = nc.dram_tensor("histo", [T, BL, H], f32, kind="ExternalOutput") if _dbg else None
    pzro = nc.dram_tensor("pzro", [64, T, PF], f32, kind="ExternalOutput") if _dbg else None
    phho = nc.dram_tensor("phho", [BL, T, PF], f32, kind="ExternalOutput") if _dbg else None
    dbgo = nc.dram_tensor("dbgo", [64, 8 * 128], f32, kind="ExternalOutput") if _dbg else None

    from contextlib import ExitStack
    with TileContext(nc) as tc, ExitStack() as _es:
        sb = _es.enter_context(tc.tile_pool(name="sb", bufs=2))
        ps = _es.enter_context(tc.tile_pool(name="ps", bufs=2, space="PSUM"))
        dr = _es.enter_context(tc.tile_pool(name="dr", bufs=1, space="DRAM"))

        hist = histo if _dbg else dr.tile([T, BL, H], f32, tag="hist", name="hist")

        # ---------------- constants ----------------
        def cload(dram, shape, tag):
            t = sb.tile(shape, f32, tag=tag, bufs=1, name=tag)
            nc.sync.dma_start(out=t[:], in_=dram[:])
            return t

        ident = cload(identd, [128, 128], "ident")
        wzx = cload(wzxd, [128, 129], "wzx")
        wrx = cload(wrxd, [128, 129], "wrx")
        whx = cload(whxd, [128, 129], "whx")
        utx = sb.tile([128, 6 * 387], f32, tag="utx", bufs=1, name="utx")
        nc.sync.dma_start(out=utx[:].rearrange("p (c n) -> p c n", c=6),
                          in_=utxd[:].rearrange("(c p) n -> p c n", p=128))
        shuf = cload(shufd, [64, 96], "shuf")   # [:,0:32]=Sr32, [:,32:96]=Sdup(rows0:32)
        bzrw = cload(bzrwd, [64, 128], "bzrw")
        bh = cload(bhd, [BL, 128], "bh")
        n2bt = cload(n2bd, [64, 1], "n2bt")
        wfuv = cload(wfuvd, [128, 128], "wfuv")
        pml = cload(pmld, [BL, C * DOUT], "pml")
        aul = cload(auld, [BL, C * DOUT], "aul")
        clc = cload(clcd, [BL, 4 * C], "clc")
        bffb = cload(bffd, [BL, DOUT], "bffb")
        bdh = cload(bdhd, [BL, DOUT], "bdh")
        m1t = cload(m1d, [BL, T], "m1t")
        m2t_ = cload(m2d, [BL, T], "m2t_")
        cidt = sb.tile([BL, 1], i32, tag="cidt", bufs=1, name="cidt")
        nc.sync.dma_start(out=cidt[:], in_=cidd[:])

        # P records resident in SBUF
        Pzr = sb.tile([64, T, PF], f32, tag="Pzr", bufs=1, name="Pzr")
        Phh = sb.tile([BL, T, PF], f32, tag="Phh", bufs=1, name="Phh")

        # ---------------- op helpers ----------------
        def tsv(out, in0, s1, s2=None, o0=AL.mult, o1=AL.bypass):
            nc.vector.tensor_scalar(out, in0, s1, s2, o0, o1)

        def ttv(out, a, b, op=AL.mult):
            nc.vector.tensor_tensor(out=out, in0=a, in1=b, op=op)

        def ttg(out, a, b, op=AL.mult):
            nc.gpsimd.tensor_tensor(out=out, in0=a, in1=b, op=op)

        def tsg(out, in0, s1, s2=None, o0=AL.mult, o1=AL.bypass):
            nc.gpsimd.tensor_scalar(out, in0, s1, s2, o0, o1)

        def stv(out, in0, s, in1, o0=AL.mult, o1=AL.add):
            nc.vector.scalar_tensor_tensor(out=out, in0=in0, scalar=s, in1=in1,
                                           op0=o0, op1=o1)

        def rcp(out, in_):
            nc.vector.reciprocal(out, in_)

        def red(out, in_):
            nc.vector.tensor_reduce(out, in_, axis=mybir.AxisListType.X, op=AL.add)

        _cbias = {}

        def cb(val, parts=128):
            v = float(val)
            if v not in _cbias:
                tname = f"cbias{len(_cbias)}"
                tcb = sb.tile([128, 1], f32, tag=tname, bufs=1, name=tname)
                nc.vector.memset(tcb[:], v)
                _cbias[v] = tcb
            return _cbias[v][0:parts, 0:1]

        def _b(bias, out):
            if isinstance(bias, (int, float)) and float(bias) not in (0.0, 1.0):
                return cb(bias, out.shape[0])
            return bias

        def asq(out, in_, scale=1.0, bias=0.0, acc=None):
            nc.scalar.activation(out, in_, AF.Square, bias=_b(bias, out), scale=scale,
                                 accum_out=acc)

        def aid(out, in_, scale=1.0, bias=0.0):
            nc.scalar.activation(out, in_, AF.Identity, bias=_b(bias, out), scale=scale)

        def acp(out, in_, scale=1.0):
            nc.scalar.activation(out, in_, AF.Copy, bias=0.0, scale=scale)

        def st(shape, tag, bufs=None):
            if bufs is None:
                bufs = 2 if (len(shape) == 2 and shape[1] <= 4) else 2
            return sb.tile(shape, f32, tag=tag, bufs=bufs, name=tag)

        # ---------------- precompute chunk (4 tokens) ----------------
        # ptile per-gate blocks of 132: [m0(128) | tau | q | pb | pp], then a
        # DRAM round-trip restacks (t,b)-partitions -> gate-stacked partitions.
        pstage = dr.tile([2, 128, 396], f32, tag="pstage", name="pstage")

        def emit_chunk(k):
            xch = sb.tile([128, DIN], f32, tag="xch", bufs=2, name="xch")
            nc.sync.dma_start(out=xch[:],
                              in_=xseq[:, 4 * k:4 * k + 4, :].rearrange("b t d -> t b d"))
            pm = ps.tile([128, 387], f32, tag="pmm", bufs=1, name="pmm")
            for j in range(6):
                pt_ps = ps.tile([128, 128], f32, tag="ptr", bufs=1, name="ptr")
                nc.tensor.transpose(out=pt_ps[:], in_=xch[:, j * 128:(j + 1) * 128],
                                    identity=ident[:])
                xt = sb.tile([128, 128], f32, tag="xt", bufs=2)
                if j % 2 == 0:
                    nc.scalar.copy(xt[:], pt_ps[:])
                else:
                    nc.vector.tensor_copy(xt[:], pt_ps[:])
                nc.tensor.matmul(out=pm[:], lhsT=xt[:], rhs=utx[:, j * 387:(j + 1) * 387],
                                 start=(j == 0), stop=(j == 5))
            ptile = sb.tile([128, 396], f32, tag="ptile", bufs=2, name="ptile")
            scr384 = sb.tile([128, 384], f32, tag="scr384", bufs=1, name="scr384")
            for g in range(3):
                c = g * 132
                if g == 1:
                    nc.vector.tensor_copy(ptile[:, c:c + 128], pm[:, g * 128:(g + 1) * 128])
                else:
                    nc.scalar.copy(ptile[:, c:c + 128], pm[:, g * 128:(g + 1) * 128])
                aid(ptile[:, c + 130:c + 131], pm[:, 384 + g:385 + g])        # pb
                ttv(scr384[:, g * 128:(g + 1) * 128], pm[:, g * 128:(g + 1) * 128],
                    ptile[:, c:c + 128])
                red(ptile[:, c + 131:c + 132], scr384[:, g * 128:(g + 1) * 128])  # pp
                y1 = sb.tile([128, 1], f32, tag="pcy1", bufs=3, name="pcy1")
                tsg(y1[:], ptile[:, c + 131:c + 132], TAU_PRE[0], TAU_PRE[1],
                    AL.mult, AL.add)
                y2 = sb.tile([128, 1], f32, tag="pcy2", bufs=3, name="pcy2")
                ttg(y2[:], y1[:], y1[:])
                tsg(ptile[:, c + 128:c + 129], y2[:], 1.0, TAU_PRE[2],
                    AL.mult, AL.add)                                           # tau
                tq_ = sb.tile([128, 1], f32, tag="pctq", bufs=3, name="pctq")
                ttg(tq_[:], ptile[:, c + 128:c + 129], ptile[:, c + 131:c + 132])
                ttg(ptile[:, c + 129:c + 130], tq_[:], ptile[:, c + 128:c + 129])  # q
            stg = pstage[k % 2]
            nc.sync.dma_start(out=stg, in_=ptile[:])
            for g in range(2):
                nc.sync.dma_start(
                    out=Pzr[g * 32:(g + 1) * 32, 4 * k:4 * k + 4, 0:132],
                    in_=stg[:, g * 132:(g + 1) * 132].rearrange("(t b) j -> b t j", b=32))
            nc.sync.dma_start(
                out=Phh[:, 4 * k:4 * k + 4, 0:132],
                in_=stg[:, 264:396].rearrange("(t b) j -> b t j", b=32))

        # ---------------- scan state ----------------
        h = st([BL, H], "h", bufs=3)
        nc.vector.memset(h[:], 0.0)
        mst = st([64, 129], "mst", bufs=3)
        nc.vector.memset(mst[:], 0.0)
        nh2 = st([BL, 1], "nh2", bufs=3)
        nc.vector.memset(nh2[:], 0.0)
        c2d = st([BL, 1], "c2d", bufs=3)
        nc.vector.memset(c2d[:], 1.0)
        sv64 = st([64, 3], "sv64", bufs=3)   # [tp0ah2 | Ah | t2ah]
        nc.vector.memset(sv64[:, 0:1], TAU_MV[0])
        nc.vector.memset(sv64[:, 1:2], 1.0)
        nc.vector.memset(sv64[:, 2:3], TAU_MV[2])

        def emit_step(t):
            nonlocal h, mst, nh2, c2d, sv64
            tauL = Pzr[:, t, 128:129]
            qL = Pzr[:, t, 129:130]
            pbL = Pzr[:, t, 130:131]
            ppL = Pzr[:, t, 131:132]
            m0L = Pzr[:, t, 0:128]
            tauH = Phh[:, t, 128:129]
            qH = Phh[:, t, 129:130]
            pbH = Phh[:, t, 130:131]
            ppH = Phh[:, t, 131:132]
            m0H = Phh[:, t, 0:128]

            # === gates: m from recurrence state ===
            pm = mst
            m2 = st([64, 1], "m2")
            scr_ = st([64, 128], "scrW64", bufs=4)
            asq(scr_[:], pm[:, 0:128], acc=m2[:])
            scrB = st([64, 128], "scrW64", bufs=4)
            ttv(scrB[:], pm[:, 0:128], m0L)
            mp = st([64, 1], "mp")
            red(mp[:], scrB[:])
            mb = pm[:, 128:129]

            # === gate mobius#1 coefs (alternate Act/DVE) ===
            ysq = st([64, 1], "ysq"); asq(ysq[:], m2[:], scale=sv64[:, 0:1], bias=TAU_MV[1])
            Czr = st([64, 1], "Czr"); tsv(Czr[:], ysq[:], sv64[:, 1:2], sv64[:, 2:3],
                                          AL.mult, AL.add)
            Czr2 = st([64, 1], "Czr2"); asq(Czr2[:], Czr[:])
            x2 = st([64, 1], "x2"); tsv(x2[:], m2[:], Czr2[:, 0:1])
            tq = st([64, 1], "tq"); aid(tq[:], tauL, scale=Czr[:, 0:1])
            xy = st([64, 1], "xy"); tsv(xy[:], mp[:], tq[:, 0:1])
            w = st([64, 1], "w"); aid(w[:], xy[:], scale=2.0, bias=1.0)
            c1 = st([64, 1], "c1"); ttv(c1[:], w[:], qL, AL.add)
            den = st([64, 1], "den"); tsv(den[:], x2[:], qL, w[:, 0:1], AL.mult, AL.add)
            rr = st([64, 1], "rr"); rcp(rr[:], den[:])
            c1r = st([64, 1], "c1r"); acp(c1r[:], c1[:], scale=rr[:, 0:1])
            C1 = st([64, 1], "C1"); tsv(C1[:], c1r[:], Czr[:, 0:1])
            c2_ = st([64, 1], "c2_"); aid(c2_[:], x2[:], scale=-1.0, bias=1.0)
            c2r = st([64, 1], "c2r"); tsv(c2r[:], c2_[:], rr[:, 0:1])
            C2t = st([64, 1], "C2t"); acp(C2t[:], c2r[:], scale=tauL)

            # scalar track: xyp, x2p
            u1 = st([64, 1], "u1"); tsv(u1[:], mb, C1[:, 0:1])
            xyp = st([64, 1], "xyp"); aid(xyp[:], pbL, scale=C2t[:, 0:1], bias=u1[:, 0:1])
            a1 = st([64, 1], "a1"); asq(a1[:], C1[:])
            b1 = st([64, 1], "b1"); tsv(b1[:], m2[:], a1[:, 0:1])
            e1_ = st([64, 1], "e1_"); tsv(e1_[:], mp[:], C1[:, 0:1], 2.0, AL.mult, AL.mult)
            f1_ = st([64, 1], "f1_"); tsv(f1_[:], ppL, C2t[:, 0:1], e1_[:, 0:1],
                                          AL.mult, AL.add)
            g1s = st([64, 1], "g1s"); acp(g1s[:], f1_[:], scale=C2t[:, 0:1])
            x2p = st([64, 1], "x2p"); ttv(x2p[:], g1s[:], b1[:], AL.add)
            # mob2 coefs
            wp = st([64, 1], "wp"); aid(wp[:], xyp[:], scale=2.0, bias=1.0)
            c1p = st([64, 1], "c1p"); ttv(c1p[:], wp[:], n2bt[:, 0:1], AL.add)
            denp = st([64, 1], "denp"); tsv(denp[:], x2p[:], n2bt[:, 0:1], wp[:, 0:1],
                                            AL.mult, AL.add)
            rrp = st([64, 1], "rrp"); rcp(rrp[:], denp[:])
            D1 = st([64, 1], "D1"); acp(D1[:], c1p[:], scale=rrp[:, 0:1])
            c2q = st([64, 1], "c2q"); aid(c2q[:], x2p[:], scale=-1.0, bias=1.0)
            D2 = st([64, 1], "D2"); tsv(D2[:], c2q[:], rrp[:, 0:1])
            # o1/o2 wides (overlap with s2 scalar track)
            t0w = st([64, 128], "t0w"); tsv(t0w[:], m0L, C2t[:, 0:1])
            o1 = st([64, 128], "o1"); stv(o1[:], pm[:, 0:128], C1[:, 0:1], t0w[:])
            v1w = st([64, 128], "v1w"); tsv(v1w[:], bzrw[:], D2[:, 0:1])
            o2w = st([64, 128], "o2w"); stv(o2w[:], o1[:], D1[:, 0:1], v1w[:])
            # s2 scalar track
            i1 = st([64, 1], "i1"); tsv(i1[:], xyp[:], D2[:, 0:1], 2.0, AL.mult, AL.mult)
            i2 = st([64, 1], "i2"); aid(i2[:], x2p[:], scale=D1[:, 0:1], bias=i1[:, 0:1])
            i3 = st([64, 1], "i3"); tsv(i3[:], i2[:], D1[:, 0:1])
            j1 = st([64, 1], "j1"); asq(j1[:], D2[:])
            j2 = st([64, 1], "j2"); tsv(j2[:], j1[:], n2bt[:, 0:1], i3[:, 0:1],
                                        AL.mult, AL.add)
            yphi = st([64, 1], "yphi"); asq(yphi[:], j2[:], scale=PHI_LOG[0],
                                            bias=PHI_LOG[1])
            Ao = st([64, 1], "Ao"); tsv(Ao[:], yphi[:], 1.0, PHI_LOG[2], AL.mult, AL.add)
            lg = st([64, 128], "lg"); tsv(lg[:], o2w[:], Ao[:, 0:1])
            # shuffle r-logits to parts 0-31 BEFORE sigmoid; z-sigmoid is off
            # the critical path (z first needed at wx2)
            rp = ps.tile([32, 128], f32, tag="rp", bufs=1, name="rp")
            nc.tensor.matmul(out=rp[:], lhsT=shuf[:, 0:32], rhs=lg[:],
                             start=True, stop=True)
            rsig = st([BL, 128], "rsig")
            nc.scalar.activation(rsig[:], rp[:], AF.Sigmoid)
            wx = st([BL, 128], "wx")
            ttv(wx[:], h[:], rsig[:])
            sr = st([BL, 1], "sr")
            scr3_ = st([BL, 128], "scrW32", bufs=4)
            asq(scr3_[:], rsig[:], acc=sr[:])
            nwx = st([BL, 1], "nwx")
            scr4_ = st([BL, 128], "scrW32", bufs=4)
            asq(scr4_[:], wx[:], acc=nwx[:])

            # psi chain (rh)
            ypsi = st([BL, 1], "ypsi"); asq(ypsi[:], sr[:], scale=PSI_G[0], bias=PSI_G[1])
            crh = st([BL, 1], "crh"); tsv(crh[:], ypsi[:], 1.0, PSI_G[2], AL.mult, AL.add)
            crh2 = st([BL, 1], "crh2"); asq(crh2[:], crh[:])
            u2 = st([BL, 1], "u2"); tsv(u2[:], nwx[:], crh2[:, 0:1])
            ytau2 = st([BL, 1], "ytau2"); asq(ytau2[:], u2[:], scale=TAU_PW[0],
                                              bias=TAU_PW[1])
            Crh = st([BL, 1], "Crh"); tsv(Crh[:], ytau2[:], TAU_PW[2], crh[:, 0:1],
                                          AL.add, AL.mult)
            Crh2 = st([BL, 1], "Crh2"); asq(Crh2[:], Crh[:])
            rh2 = st([BL, 1], "rh2"); tsv(rh2[:], nwx[:], Crh2[:, 0:1])
            yphi2 = st([BL, 1], "yphi2"); asq(yphi2[:], rh2[:], scale=PHI_ST[0],
                                              bias=PHI_ST[1])
            Arh = st([BL, 1], "Arh"); tsv(Arh[:], yphi2[:], 1.0, PHI_ST[2], AL.mult, AL.add)
            arh2 = st([BL, 1], "arh2"); asq(arh2[:], Arh[:])

            # h matmul on wx
            ptp = ps.tile([128, BL], f32, tag="ptp", bufs=1, name="ptp")
            nc.tensor.transpose(out=ptp[:], in_=wx[:], identity=ident[:BL, :BL])
            rhT = st([128, BL], "rhT")
            nc.vector.tensor_copy(rhT[:], ptp[:])
            psh = ps.tile([BL, 129], f32, tag="ph", bufs=1, name="ph")
            nc.tensor.matmul(out=psh[:], lhsT=rhT[:], rhs=whx[:], start=True, stop=True)
            m2h = st([BL, 1], "m2h")
            scr5_ = st([BL, 128], "scrW32", bufs=4)
            asq(scr5_[:], psh[:, 0:128], acc=m2h[:])
            scrG = st([BL, 128], "scrW32", bufs=4)
            ttv(scrG[:], psh[:, 0:128], m0H)
            mph = st([BL, 1], "mph")
            red(mph[:], scrG[:])
            mbh = st([BL, 1], "mbh")
            aid(mbh[:], psh[:, 128:129])

            zsig = st([BL, 128], "zsig")
            nc.scalar.activation(zsig[:], lg[0:32, :], AF.Sigmoid)
            sz = st([BL, 1], "sz")
            scr2_ = st([BL, 128], "scrW32", bufs=4)
            asq(scr2_[:], zsig[:], acc=sz[:])

            # h-lane mobius#1 coefs
            arhc = st([BL, 1], "arhc"); tsv(arhc[:], arh2[:], Crh2[:, 0:1])
            uh = st([BL, 1], "uh"); acp(uh[:], m2h[:], scale=arhc[:, 0:1])
            ytauh = st([BL, 1], "ytauh"); asq(ytauh[:], uh[:], scale=TAU_MV[0],
                                              bias=TAU_MV[1])
            Czh0 = st([BL, 1], "Czh0"); tsv(Czh0[:], ytauh[:], TAU_MV[2], Arh[:, 0:1],
                                            AL.add, AL.mult)
            Czh = st([BL, 1], "Czh"); acp(Czh[:], Czh0[:], scale=Crh[:, 0:1])
            Czh2 = st([BL, 1], "Czh2"); asq(Czh2[:], Czh[:])
            x2h = st([BL, 1], "x2h"); tsv(x2h[:], m2h[:], Czh2[:, 0:1])
            tqh = st([BL, 1], "tqh"); acp(tqh[:], tauH, scale=Czh[:, 0:1])
            xyh = st([BL, 1], "xyh"); tsv(xyh[:], mph[:], tqh[:, 0:1])
            wh_ = st([BL, 1], "wh_"); aid(wh_[:], xyh[:], scale=2.0, bias=1.0)
            c1h = st([BL, 1], "c1h"); ttv(c1h[:], wh_[:], qH, AL.add)
            denh = st([BL, 1], "denh"); tsv(denh[:], x2h[:], qH, wh_[:, 0:1],
                                            AL.mult, AL.add)
            rrh = st([BL, 1], "rrh"); rcp(rrh[:], denh[:])
            c1rh = st([BL, 1], "c1rh"); acp(c1rh[:], c1h[:], scale=rrh[:, 0:1])
            C1h = st([BL, 1], "C1h"); tsv(C1h[:], c1rh[:], Czh[:, 0:1])
            c2h2 = st([BL, 1], "c2h2"); aid(c2h2[:], x2h[:], scale=-1.0, bias=1.0)
            c2rh = st([BL, 1], "c2rh"); tsv(c2rh[:], c2h2[:], rrh[:, 0:1])
            C2th = st([BL, 1], "C2th"); acp(C2th[:], c2rh[:], scale=tauH)

            # scalar track: xyph, x2ph
            u1h = st([BL, 1], "u1h"); tsv(u1h[:], mbh[:], C1h[:, 0:1])
            xyph = st([BL, 1], "xyph"); aid(xyph[:], pbH, scale=C2th[:, 0:1],
                                            bias=u1h[:, 0:1])
            a1h = st([BL, 1], "a1h"); asq(a1h[:], C1h[:])
            b1h = st([BL, 1], "b1h"); tsv(b1h[:], m2h[:], a1h[:, 0:1])
            e1h = st([BL, 1], "e1h"); tsv(e1h[:], mph[:], C1h[:, 0:1], 2.0, AL.mult, AL.mult)
            f1h = st([BL, 1], "f1h"); tsv(f1h[:], ppH, C2th[:, 0:1], e1h[:, 0:1],
                                          AL.mult, AL.add)
            g1h = st([BL, 1], "g1h"); acp(g1h[:], f1h[:], scale=C2th[:, 0:1])
            x2ph = st([BL, 1], "x2ph"); ttv(x2ph[:], g1h[:], b1h[:], AL.add)
            # h-lane bias mobius coefs
            n2bh = host["n2bh"]
            wph = st([BL, 1], "wph"); aid(wph[:], xyph[:], scale=2.0, bias=1.0)
            c1ph = st([BL, 1], "c1ph"); tsv(c1ph[:], wph[:], n2bh, None, AL.add)
            denph = st([BL, 1], "denph"); tsv(denph[:], x2ph[:], n2bh, wph[:, 0:1],
                                              AL.mult, AL.add)
            rrph = st([BL, 1], "rrph"); rcp(rrph[:], denph[:])
            D1h = st([BL, 1], "D1h"); acp(D1h[:], c1ph[:], scale=rrph[:, 0:1])
            c2pph = st([BL, 1], "c2pph"); aid(c2pph[:], x2ph[:], scale=-1.0, bias=1.0)
            D2h = st([BL, 1], "D2h"); tsv(D2h[:], c2pph[:], rrph[:, 0:1])
            G1 = st([BL, 1], "G1"); acp(G1[:], D1h[:], scale=C1h[:, 0:1])
            G2 = st([BL, 1], "G2"); tsv(G2[:], D1h[:], C2th[:, 0:1])

            # ht wides
            tp2 = st([BL, 128], "tp2"); tsv(tp2[:], m0H, G2[:, 0:1])
            tm2 = st([BL, 128], "tm2"); stv(tm2[:], psh[:, 0:128], G1[:, 0:1], tp2[:])
            ht = st([BL, 128], "ht")
            t2w = st([BL, 128], "t2w"); acp(t2w[:], bh[:], scale=D2h[:, 0:1])
            ttv(ht[:], t2w[:], tm2[:], AL.add)

            # === delta ===
            y2d = st([BL, 1], "y2d")
            scr6_ = st([BL, 128], "scrW32", bufs=4)
            asq(scr6_[:], ht[:], acc=y2d[:])
            scrI = st([BL, 128], "scrW32", bufs=4)
            ttv(scrI[:], h[:], ht[:])
            xyd = st([BL, 1], "xyd")
            red(xyd[:], scrI[:])
            wd = st([BL, 1], "wd"); aid(wd[:], xyd[:], scale=-2.0, bias=1.0)
            c1d = st([BL, 1], "c1d"); ttv(c1d[:], wd[:], y2d[:], AL.add)
            dend = st([BL, 1], "dend"); tsv(dend[:], y2d[:], nh2[:, 0:1], wd[:, 0:1],
                                            AL.mult, AL.add)
            rrd = st([BL, 1], "rrd"); rcp(rrd[:], dend[:])
            nCd1 = st([BL, 1], "nCd1"); tsv(nCd1[:], c1d[:], rrd[:, 0:1], -1.0,
                                            AL.mult, AL.mult)
            Cd2 = st([BL, 1], "Cd2"); acp(Cd2[:], c2d[:], scale=rrd[:, 0:1])
            td = st([BL, 128], "td"); tsv(td[:], ht[:], Cd2[:, 0:1])
            delta = st([BL, 128], "delta")
            stv(delta[:], h[:], nCd1[:, 0:1], td[:])

            # === pointwise z + h_new ===
            wx2 = st([BL, 128], "wx2"); ttv(wx2[:], delta[:], zsig[:])
            ptq = ps.tile([128, BL], f32, tag="ptp", bufs=1, name="ptp")
            nc.tensor.transpose(out=ptq[:], in_=wx2[:], identity=ident[:BL, :BL])
            qT = st([128, 64], "qT")
            nc.scalar.copy(qT[:, 0:32], ptq[:])
            nc.vector.tensor_copy(qT[:, 32:64], ptq[:])
            qm = ps.tile([64, 129], f32, tag="pg", bufs=1, name="pg")
            nc.tensor.matmul(out=qm[0:32, :], lhsT=qT[:, 0:32], rhs=wzx[:],
                             start=True, stop=True)
            nc.tensor.matmul(out=qm[32:64, :], lhsT=qT[:, 32:64], rhs=wrx[:],
                             start=True, stop=True)
            nwx2 = st([BL, 1], "nwx2")
            scr7_ = st([BL, 128], "scrW32", bufs=4)
            asq(scr7_[:], wx2[:], acc=nwx2[:])
            scrK = st([BL, 128], "scrW32", bufs=4)
            ttv(scrK[:], h[:], wx2[:])
            xyp2 = st([BL, 1], "xyp2")
            red(xyp2[:], scrK[:])
            ypsi2 = st([BL, 1], "ypsi2"); asq(ypsi2[:], sz[:], scale=PSI_G[0],
                                              bias=PSI_G[1])
            czp = st([BL, 1], "czp"); tsv(czp[:], ypsi2[:], 1.0, PSI_G[2], AL.mult, AL.add)
            cz2p = st([BL, 1], "cz2p"); asq(cz2p[:], czp[:])
            u3 = st([BL, 1], "u3"); tsv(u3[:], nwx2[:], cz2p[:, 0:1])
            ytau3 = st([BL, 1], "ytau3"); asq(ytau3[:], u3[:], scale=TAU_PW[0],
                                              bias=TAU_PW[1])
            Cpw = st([BL, 1], "Cpw"); tsv(Cpw[:], ytau3[:], TAU_PW[2], czp[:, 0:1],
                                          AL.add, AL.mult)
            Cpw2 = st([BL, 1], "Cpw2"); asq(Cpw2[:], Cpw[:])
            y2n = st([BL, 1], "y2n"); tsv(y2n[:], nwx2[:], Cpw2[:, 0:1])
            xyn = st([BL, 1], "xyn"); tsv(xyn[:], xyp2[:], Cpw[:, 0:1])
            wn = st([BL, 1], "wn"); aid(wn[:], xyn[:], scale=2.0, bias=1.0)
            c1n = st([BL, 1], "c1n"); ttv(c1n[:], wn[:], y2n[:], AL.add)
            denn = st([BL, 1], "denn"); tsv(denn[:], y2n[:], nh2[:, 0:1], wn[:, 0:1],
                                            AL.mult, AL.add)
            rrn = st([BL, 1], "rrn"); rcp(rrn[:], denn[:])
            sv2 = st([BL, 2], "sv2", bufs=3)
            acp(sv2[:, 0:1], c1n[:], scale=rrn[:, 0:1])
            C1n = sv2[:, 0:1]
            C2n = st([BL, 1], "C2n"); tsv(C2n[:], c2d[:], rrn[:, 0:1])
            acp(sv2[:, 1:2], C2n[:], scale=Cpw[:, 0:1])
            C2nw = sv2[:, 1:2]
            sv2p = ps.tile([64, 2], f32, tag="svp2", bufs=1, name="svp2")
            nc.tensor.matmul(out=sv2p[:], lhsT=shuf[0:32, 32:96], rhs=sv2[:],
                             start=True, stop=True)
            sv2d = st([64, 2], "sv2d")
            nc.vector.tensor_copy(sv2d[:], sv2p[:])
            tn = st([BL, 128], "tn"); tsv(tn[:], wx2[:], C2nw)
            h_new = st([BL, H], "h", bufs=3)
            stv(h_new[:], h[:], C1n, tn[:])
            nc.sync.dma_start(out=hist[t], in_=h_new[:])

            if _dbg and t == 0:
                nc.sync.dma_start(out=dbgo[:, 0 * 128:1 * 128], in_=zr[:])
                nc.sync.dma_start(out=dbgo[0:32, 1 * 128:2 * 128], in_=ht[:])
                nc.sync.dma_start(out=dbgo[0:32, 2 * 128:3 * 128], in_=delta[:])
                nc.sync.dma_start(out=dbgo[0:32, 3 * 128:4 * 128], in_=wx2[:])
                nc.sync.dma_start(out=dbgo[:, 4 * 128:5 * 128], in_=o2w[:])
                nc.sync.dma_start(out=dbgo[:, 5 * 128:6 * 128], in_=lg[:])
                nc.sync.dma_start(out=dbgo[0:32, 6 * 128:7 * 128], in_=h_new[:])
            # === finalize: state scalars + transposes ===
            k1 = st([BL, 1], "k1"); asq(k1[:], C1n)
            k2 = st([BL, 1], "k2"); tsv(k2[:], k1[:], nh2[:, 0:1])
            k3 = st([BL, 1], "k3"); tsv(k3[:], C1n, C2nw)
            k4 = st([BL, 1], "k4"); acp(k4[:], xyp2[:], scale=k3[:, 0:1])
            k6 = st([BL, 1], "k6"); asq(k6[:], C2nw)
            k7 = st([BL, 1], "k7"); tsv(k7[:], k6[:], nwx2[:, 0:1])
            k8 = st([BL, 1], "k8"); aid(k8[:], k4[:], scale=2.0, bias=k2[:, 0:1])
            nh2n = st([BL, 1], "nh2", bufs=3); ttv(nh2n[:], k8[:], k7[:], AL.add)
            c2dn = st([BL, 1], "c2d", bufs=3); aid(c2dn[:], nh2n[:], scale=-1.0, bias=1.0)
            sv = st([BL, 3], "sv", bufs=3)
            yphin = st([BL, 1], "yphin"); asq(yphin[:], nh2n[:], scale=PHI_ST[0],
                                              bias=PHI_ST[1])
            tsv(sv[:, 1:2], yphin[:], 1.0, PHI_ST[2], AL.mult, AL.add)       # Ah
            ah2n = st([BL, 1], "ah2n"); asq(ah2n[:], sv[:, 1:2])
            tsv(sv[:, 0:1], ah2n[:], TAU_MV[0])                              # tp0ah2
            aid(sv[:, 2:3], sv[:, 1:2], scale=TAU_MV[2])                     # t2ah
            svp = ps.tile([64, 3], f32, tag="svp", bufs=1, name="svp")
            nc.tensor.matmul(out=svp[:], lhsT=shuf[0:32, 32:96], rhs=sv[:],
                             start=True, stop=True)
            sv64n = st([64, 3], "sv64", bufs=3)
            nc.vector.tensor_copy(sv64n[:], svp[:])
            # m recurrence: m_new = C1n*m_prev + C2nw*(wx2 @ Wg^T)
            u_m = st([64, 129], "u_m")
            tsv(u_m[:], qm[:], sv2d[:, 1:2])
            mnew = st([64, 129], "mst", bufs=3)
            stv(mnew[:], mst[:], sv2d[:, 0:1], u_m[:])

            h, mst, nh2, c2d, sv64 = h_new, mnew, nh2n, c2dn, sv64n

        # ---------------- emit ----------------
        for k in range(3):
            emit_chunk(k)
        for t in range(T):
            if t % 4 == 0 and (t // 4 + 3) < 32:
                emit_chunk(t // 4 + 3)
            emit_step(t)

        if _dbg:
            nc.sync.dma_start(out=pzro[:], in_=Pzr[:])
            nc.sync.dma_start(out=phho[:], in_=Phh[:])
        # ================= head (ported from baseline) =================
        iotaT = sb.tile([BL, T], i32, tag="iotaT", bufs=1)
        nc.gpsimd.iota(iotaT[:], pattern=[[1, T]], base=0, channel_multiplier=0)
        iotaTf = st([BL, T], "iotaTf", bufs=1)
        nc.vector.tensor_copy(iotaTf[:], iotaT[:])
        bidx = sb.tile([BL, 1], i32, tag="bidx", bufs=1, name="bidx")
        nc.gpsimd.iota(bidx[:], pattern=[[0, 1]], base=0, channel_multiplier=1)
        bidxf = st([BL, 1], "bidxf", bufs=1)
        nc.vector.tensor_copy(bidxf[:], bidx[:])

        def ttr(scr_, a, b, acc):
            nc.vector.tensor_tensor(out=scr_, in0=a, in1=b, op=AL.mult)
            nc.vector.tensor_reduce(acc, scr_, axis=mybir.AxisListType.X, op=AL.add)

        uv = st([BL, 256], "uv", bufs=1)
        for i, mt in enumerate((m1t, m2t_)):
            pos = st([BL, 1], f"pos{i}", bufs=1)
            scr_ = st([BL, 128], "scr", bufs=1)
            ttr(scr_[:], mt[:], iotaTf[:], pos[:])
            ridf = st([BL, 1], f"ridf{i}", bufs=1)
            stv(ridf[:], pos[:], float(BL), bidxf[:])
            ridi = sb.tile([BL, 1], i32, tag=f"ridi{i}", bufs=1)
            nc.vector.tensor_copy(ridi[:], ridf[:])
            nc.gpsimd.indirect_dma_start(
                out=uv[:, i * 128:(i + 1) * 128], out_offset=None,
                in_=hist[:].rearrange("t b h -> (t b) h"),
                in_offset=bass.IndirectOffsetOnAxis(ap=ridi[:, 0:1], axis=0))

        u_ap = uv[:, 0:128]; v_ap = uv[:, 128:256]

        x2u = st([BL, 1], "x2u", bufs=1)
        scr_ = st([BL, 128], "scr", bufs=1); asq(scr_[:], u_ap, acc=x2u[:])
        y2v = st([BL, 1], "y2v", bufs=1)
        scr_ = st([BL, 128], "scr", bufs=1); asq(scr_[:], v_ap, acc=y2v[:])
        xyuv = st([BL, 1], "xyuv", bufs=1)
        scr_ = st([BL, 128], "scr", bufs=1); ttr(scr_[:], u_ap, v_ap, xyuv[:])

        wuv = st([BL, 1], "wuv", bufs=1); aid(wuv[:], xyuv[:], scale=-2.0, bias=1.0)
        c1uv = st([BL, 1], "c1uv", bufs=1); tsv(c1uv[:], wuv[:], y2v[:, 0:1], None, AL.add)
        denuv = st([BL, 1], "denuv", bufs=1); tsv(denuv[:], y2v[:], x2u[:, 0:1],
                                                  wuv[:, 0:1], AL.mult, AL.add)
        ruv = st([BL, 1], "ruv", bufs=1); rcp(ruv[:], denuv[:])
        Cu1 = st([BL, 1], "Cu1", bufs=1); acp(Cu1[:], c1uv[:], scale=ruv[:, 0:1])
        Cu2t = st([BL, 1], "Cu2t", bufs=1); tsv(Cu2t[:], x2u[:], -1.0, 1.0, AL.mult, AL.add)
        Cu2 = st([BL, 1], "Cu2", bufs=1); acp(Cu2[:], Cu2t[:], scale=ruv[:, 0:1])
        q1 = st([BL, 1], "q1", bufs=1); asq(q1[:], Cu1[:])
        q1x = st([BL, 1], "q1x", bufs=1); tsv(q1x[:], q1[:], x2u[:, 0:1])
        q2 = st([BL, 1], "q2", bufs=1); tsv(q2[:], Cu1[:], Cu2[:, 0:1])
        q2x = st([BL, 1], "q2x", bufs=1); tsv(q2x[:], q2[:], xyuv[:, 0:1])
        q3 = st([BL, 1], "q3", bufs=1); asq(q3[:], Cu2[:])
        q3x = st([BL, 1], "q3x", bufs=1); tsv(q3x[:], q3[:], y2v[:, 0:1])
        nd2 = st([BL, 1], "nd2", bufs=1)
        stv(nd2[:], q2x[:], -2.0, q1x[:])
        ttv(nd2[:], nd2[:], q3x[:], AL.add)
        nd = st([BL, 1], "nd", bufs=1)
        nc.scalar.activation(nd[:], nd2[:], AF.Sqrt)
        yph = st([BL, 1], "yph", bufs=1); asq(yph[:], nd2[:], scale=PHI_HEAD[0],
                                              bias=PHI_HEAD[1])
        phih = st([BL, 1], "phih", bufs=1); tsv(phih[:], yph[:], PHI_HEAD[2], None, AL.add)
        dsq0 = st([BL, 1], "dsq0", bufs=1); tsv(dsq0[:], nd[:], phih[:, 0:1])
        yk = st([BL, 1], "yk", bufs=1)
        tsv(yk[:], dsq0[:], host["k02"])
        yk2 = st([BL, 1], "yk2", bufs=1); asq(yk2[:], yk[:])
        yk4 = st([BL, 1], "yk4", bufs=1); asq(yk4[:], yk2[:])
        tser = st([BL, 1], "tser", bufs=1)
        tsv(tser[:], yk2[:], float(-1.0 / 3.0), 1.0, AL.mult, AL.add)
        stv(tser[:], yk4[:], float(2.0 / 15.0), tser[:])
        tk = st([BL, 1], "tk", bufs=1); tsv(tk[:], tser[:], yk[:, 0:1])

        ptu = ps.tile([128, BL], f32, tag="ptp", bufs=1, name="ptp")
        nc.tensor.transpose(out=ptu[:], in_=u_ap, identity=ident[:BL, :BL])
        uT = st([128, BL], "uT", bufs=1); nc.scalar.copy(uT[:], ptu[:])
        ptv = ps.tile([128, BL], f32, tag="ptp", bufs=1, name="ptp")
        nc.tensor.transpose(out=ptv[:], in_=v_ap, identity=ident[:BL, :BL])
        vT = st([128, BL], "vT", bufs=1); nc.scalar.copy(vT[:], ptv[:])
        psu = ps.tile([BL, DOUT], f32, tag="pg", bufs=1, name="pg")
        nc.tensor.matmul(out=psu[:], lhsT=uT[:], rhs=wfuv[:, 0:64], start=True, stop=True)
        psv = ps.tile([BL, DOUT], f32, tag="ph", bufs=1, name="ph")
        nc.tensor.matmul(out=psv[:], lhsT=vT[:], rhs=wfuv[:, 64:128], start=True, stop=True)

        def head_mvec(psx, sx, tag):
            ya = st([BL, 1], f"ya{tag}", bufs=1); asq(ya[:], sx[:], scale=PHI_ST[0],
                                                      bias=PHI_ST[1])
            a_ = st([BL, 1], f"a{tag}", bufs=1); tsv(a_[:], ya[:], PHI_ST[2], None, AL.add)
            n2_ = st([BL, 1], f"n2{tag}", bufs=1)
            scr_ = st([BL, 128], "scr", bufs=1)
            asq(scr_[:, 0:DOUT], psx[:], acc=n2_[:])
            a2_ = st([BL, 1], f"aa{tag}", bufs=1); asq(a2_[:], a_[:])
            uu_ = st([BL, 1], f"uu{tag}", bufs=1); tsv(uu_[:], a2_[:], n2_[:, 0:1])
            yt_ = st([BL, 1], f"yt{tag}", bufs=1); asq(yt_[:], uu_[:], scale=TAU_MV[0],
                                                       bias=TAU_MV[1])
            cf_ = st([BL, 1], f"cf{tag}", bufs=1); tsv(cf_[:], yt_[:], TAU_MV[2],
                                                       a_[:, 0:1], AL.add, AL.mult)
            mx = st([BL, DOUT], f"mx{tag}", bufs=1)
            tsv(mx[:], psx[:], cf_[:, 0:1])
            n2o = st([BL, 1], f"n2o{tag}", bufs=1)
            cf2 = st([BL, 1], f"cf2{tag}", bufs=1); asq(cf2[:], cf_[:])
            tsv(n2o[:], cf2[:], n2_[:, 0:1])
            return mx, n2o

        mu, n2mu = head_mvec(psu, x2u, "u")
        mv, n2mv = head_mvec(psv, y2v, "v")

        def head_mob_add(x_ap, x2_ap, y_ap, y2_ap, tag, y2_imm=None):
            xy_ = st([BL, 1], f"hxy{tag}", bufs=1)
            scr_ = st([BL, 128], "scr", bufs=1)
            ttr(scr_[:, 0:DOUT], x_ap, y_ap, xy_[:])
            w_ = st([BL, 1], f"hw{tag}", bufs=1); aid(w_[:], xy_[:], scale=2.0, bias=1.0)
            c1_ = st([BL, 1], f"hc1{tag}", bufs=1)
            dn_ = st([BL, 1], f"hdn{tag}", bufs=1)
            if y2_imm is not None:
                tsv(c1_[:], w_[:], y2_imm, None, AL.add)
                tsv(dn_[:], x2_ap, y2_imm, w_[:, 0:1], AL.mult, AL.add)
            else:
                tsv(c1_[:], w_[:], y2_ap[:, 0:1], None, AL.add)
                tsv(dn_[:], x2_ap, y2_ap[:, 0:1], w_[:, 0:1], AL.mult, AL.add)
            rr_ = st([BL, 1], f"hrr{tag}", bufs=1); rcp(rr_[:], dn_[:])
            A1 = st([BL, 1], f"hA1{tag}", bufs=1); acp(A1[:], c1_[:], scale=rr_[:, 0:1])
            c2t_ = st([BL, 1], f"hc2{tag}", bufs=1); tsv(c2t_[:], x2_ap, -1.0, 1.0,
                                                         AL.mult, AL.add)
            A2 = st([BL, 1], f"hA2{tag}", bufs=1); acp(A2[:], c2t_[:], scale=rr_[:, 0:1])
            t_ = st([BL, DOUT], f"hT{tag}", bufs=1)
            tsv(t_[:], y_ap, A2[:, 0:1])
            o_ = st([BL, DOUT], f"ho{tag}", bufs=1)
            stv(o_[:], x_ap, A1[:, 0:1], t_[:])
            so_ = st([BL, 1], f"hso{tag}", bufs=1)
            scr2_ = st([BL, 128], "scr", bufs=1)
            asq(scr2_[:, 0:DOUT], o_[:], acc=so_[:])
            return o_, so_

        o_, so_ = head_mob_add(mu[:], n2mu[:, 0:1], mv[:], n2mv, "a")
        o_, so_ = head_mob_add(o_[:], so_[:, 0:1], bffb[:], None, "b", y2_imm=host["n2bff"])
        sm = st([BL, DOUT], "sm", bufs=1); tsv(sm[:], bdh[:], tk[:, 0:1])
        n2sm = st([BL, 1], "n2sm", bufs=1); asq(n2sm[:], tk[:])
        o_, so_ = head_mob_add(o_[:], so_[:, 0:1], sm[:], n2sm, "c")
        ct = st([BL, DOUT], "ct", bufs=1)
        nc.gpsimd.indirect_dma_start(
            out=ct[:], out_offset=None, in_=ctabd[:],
            in_offset=bass.IndirectOffsetOnAxis(ap=cidt[:, 0:1], axis=0))
        n2ct = st([BL, 1], "n2ct", bufs=1)
        scr_ = st([BL, 128], "scr", bufs=1)
        asq(scr_[:, 0:DOUT], ct[:], acc=n2ct[:])
        o_, so_ = head_mob_add(o_[:], so_[:, 0:1], ct[:], n2ct, "d")

        pout = st([BL, C], "pout", bufs=1)
        aout = st([BL, C], "aout", bufs=1)
        for c in range(C):
            scr_ = st([BL, 128], "scr", bufs=1)
            ttr(scr_[:, 0:DOUT], o_[:], pml[:, c * DOUT:(c + 1) * DOUT], pout[:, c:c + 1])
            scr2_ = st([BL, 128], "scr", bufs=1)
            ttr(scr2_[:, 0:DOUT], o_[:], aul[:, c * DOUT:(c + 1) * DOUT], aout[:, c:c + 1])
        ppc = clc[:, 0:C]; pac = clc[:, C:2 * C]; naf = clc[:, 2 * C:3 * C]
        ompc = clc[:, 3 * C:4 * C]
        wm = st([BL, C], "wm", bufs=1); aid(wm[:], pout[:], scale=-2.0, bias=1.0)
